# revision 18
# baseline (speedup 1.0000x reference)
"""GroupedQueryAttention on 8 Trainium2 NeuronCores (Bass/Tile).

Tensor-parallel over heads: core c owns q-heads 4c..4c+3 and kv-head c.
Per core: bf16 projections + on-chip interleaved RoPE (pair-swap via a
permutation matmul), causal attention per 256-row q-block (softmax without
max-subtraction; denominator via a ones-column in the PV matmul), then an
AllGather of y^T and a transposed out-projection producing the core's
256-column slice of the output (host re-transposes and concatenates).

Attention matmuls process head-pairs (N=512 moving operand) and share the
stationary k/v tiles; softmax exp runs on ScalarE in [128,4,256] batches.
"""
import os
import sys
import types

os.environ.setdefault("JAX_PLATFORMS", "cpu,axon")

import numpy as np
import ml_dtypes

BF = ml_dtypes.bfloat16

# Optional NTFF-profile hook injection (lets BASS_TRACE=1 capture exec_time).
try:
    import antenv.axon_hooks  # noqa: F401
except ImportError:
    try:
        _hm = types.ModuleType("antenv.axon_hooks")
        _hs = [None]
        _hm.set_axon_ntff_profile_hook = lambda h: _hs.__setitem__(0, h)
        _hm.get_axon_ntff_profile_hook = lambda: _hs[0]
        sys.modules["antenv.axon_hooks"] = _hm
        import antenv

        antenv.axon_hooks = _hm
        from trn_agent_boot.trn_boot import _ntff_profile_via_ctypes

        _hook = _ntff_profile_via_ctypes("/opt/axon/libaxon_pjrt.so")
        if _hook is not None:
            _hm.set_axon_ntff_profile_hook(_hook)
    except Exception:
        pass

import concourse.bass as bass
import concourse.tile as tile
from concourse import bacc, mybir
from concourse.bass_utils import run_bass_kernel_spmd

B, T, DIM = 2, 2048, 2048
N_HEADS, N_KV_HEADS, HEAD_DIM = 32, 8, 64
NCORES = 8
HPC = N_HEADS // NCORES  # 4 q heads per core
DT = mybir.dt.bfloat16
F32 = mybir.dt.float32

LAST_RESULTS = None  # BassKernelResults of the most recent run (for test.py)


def build_nc(Tt=T):
    """Build + compile the SPMD program (same for all 8 cores)."""
    BT = B * Tt
    PQB = Tt // 256  # q-blocks per batch
    NCH = BT // 512  # all-gather chunks
    NTT = BT // 128  # 128-token tiles
    NGG = BT // 512  # x streaming groups
    assert BT % 1024 == 0

    nc = bacc.Bacc("TRN2", target_bir_lowering=False, debug=False,
                   num_devices=NCORES)

    xT = nc.declare_dram_parameter("xT", [DIM, BT], DT, isOutput=False)
    wqT = nc.declare_dram_parameter("wqT", [DIM, 256], DT, isOutput=False)
    wkvT = nc.declare_dram_parameter("wkvT", [DIM, 128], DT, isOutput=False)
    woT = nc.declare_dram_parameter("woT", [DIM, 256], DT, isOutput=False)
    cos2 = nc.declare_dram_parameter("cos2", [128, BT], DT, isOutput=False)
    sin2 = nc.declare_dram_parameter("sin2", [128, BT], DT, isOutput=False)
    pswp = nc.declare_dram_parameter("pswp", [128, 128], DT, isOutput=False)
    ident = nc.declare_dram_parameter("ident", [64, 64], DT, isOutput=False)
    mask0 = nc.declare_dram_parameter("mask0", [128, 256], DT, isOutput=False)
    mask1 = nc.declare_dram_parameter("mask1", [128, 256], DT, isOutput=False)
    outp = nc.declare_dram_parameter("out", [256, BT], F32, isOutput=True)

    with tile.TileContext(nc) as tc:
        with tc.tile_pool(name="persist", bufs=1) as persist, \
             tc.tile_pool(name="dram", bufs=1, space="DRAM") as dramp:
            qT_sb = persist.tile([128, HPC, BT], DT)
            kT_sb = persist.tile([128, BT], DT)
            v_sb = persist.tile([128, NTT, 65], DT)
            yloc = persist.tile([64, HPC, BT], DT)
            wo_sb = persist.tile([128, 16, 256], DT)
            pswp_sb = persist.tile([128, 128], DT)
            id_sb = persist.tile([64, 64], DT)
            m0_sb = persist.tile([128, 256], DT)
            m1_sb = persist.tile([128, 256], DT)

            ytloc = [dramp.tile([256, 512], DT, name=f"ytloc{c}")
                     for c in range(NCH)]
            ytful = [dramp.tile([2048, 512], DT, addr_space="Shared",
                                name=f"ytful{c}") for c in range(NCH)]

            # ---------------- phase 1: projections + RoPE ----------------
            with tc.tile_pool(name="wpool", bufs=1) as wpool, \
                 tc.tile_pool(name="xs", bufs=2) as xs, \
                 tc.tile_pool(name="cs", bufs=2) as cs, \
                 tc.tile_pool(name="ptmp", bufs=3) as ptmp, \
                 tc.tile_pool(name="ps_mm", bufs=2, space="PSUM") as ps_mm, \
                 tc.tile_pool(name="ps_sw", bufs=2, space="PSUM") as ps_sw, \
                 tc.tile_pool(name="ps_vt", bufs=2, space="PSUM") as ps_vt:
                wq_sb = wpool.tile([128, 16, 256], DT)
                wkv_sb = wpool.tile([128, 16, 128], DT)
                for ct in range(16):
                    nc.sync.dma_start(out=wq_sb[:, ct, :],
                                      in_=wqT[128 * ct:128 * ct + 128, :])
                    nc.sync.dma_start(out=wkv_sb[:, ct, :],
                                      in_=wkvT[128 * ct:128 * ct + 128, :])
                nc.sync.dma_start(out=pswp_sb, in_=pswp[:])
                nc.sync.dma_start(out=id_sb, in_=ident[:])
                nc.sync.dma_start(out=m0_sb, in_=mask0[:])
                nc.sync.dma_start(out=m1_sb, in_=mask1[:])
                nc.vector.memset(v_sb[:, :, 64:65], 1.0)
                nc.vector.memset(qT_sb[64:128, :, :], 0.0)
                nc.vector.memset(kT_sb[64:128, :], 0.0)

                for gg in range(NGG):
                    c0 = 512 * gg
                    xt = xs.tile([128, 16, 512], DT, tag="xt")
                    for ct in range(16):
                        nc.gpsimd.dma_start(
                            out=xt[:, ct, :],
                            in_=xT[128 * ct:128 * ct + 128, c0:c0 + 512])
                    cost = cs.tile([128, 512], DT, tag="cost")
                    sint = cs.tile([128, 512], DT, tag="sint")
                    nc.sync.dma_start(out=cost, in_=cos2[:, c0:c0 + 512])
                    nc.sync.dma_start(out=sint, in_=sin2[:, c0:c0 + 512])

                    # m = 0,1: q head-pairs; m = 2: kv
                    for m in range(3):
                        pmm = ps_mm.tile([128, 512], F32, tag="mm")
                        for ct in range(16):
                            if m < 2:
                                w_ap = wq_sb[:, ct, 128 * m:128 * m + 128]
                            else:
                                w_ap = wkv_sb[:, ct, :]
                            nc.tensor.matmul(
                                pmm, lhsT=w_ap, rhs=xt[:, ct, :],
                                start=(ct == 0), stop=(ct == 15))
                        for sub in range(1):
                            cols = slice(c0, c0 + 512)
                            lc = slice(0, 512)
                            if m < 2:
                                qraw = ptmp.tile([128, 512], DT, tag="qraw")
                                nc.scalar.copy(qraw, pmm)
                                psw = ps_sw.tile([128, 512], F32, tag="sw")
                                nc.tensor.matmul(psw, lhsT=pswp_sb, rhs=qraw,
                                                 start=True, stop=True)
                                qsw = ptmp.tile([128, 512], DT, tag="qsw")
                                nc.scalar.copy(qsw, psw)
                                for hh in range(2):
                                    h = 2 * m + hh
                                    rows = slice(64 * hh, 64 * hh + 64)
                                    t0 = ptmp.tile([64, 512], DT, tag="t0")
                                    t1 = ptmp.tile([64, 512], DT, tag="t1")
                                    nc.vector.tensor_mul(
                                        t0, qraw[rows, :], cost[rows, :])
                                    nc.vector.tensor_mul(
                                        t1, qsw[rows, :], sint[rows, :])
                                    nc.vector.tensor_add(
                                        qT_sb[0:64, h, cols], t0, t1)
                            else:
                                kraw = ptmp.tile([64, 512], DT, tag="kraw")
                                vraw = ptmp.tile([64, 512], DT, tag="vraw")
                                nc.scalar.copy(kraw, pmm[0:64, :])
                                nc.scalar.copy(vraw, pmm[64:128, :])
                                psw = ps_sw.tile([128, 512], F32, tag="sw")
                                nc.tensor.matmul(
                                    psw[0:64, :], lhsT=pswp_sb[0:64, 0:64],
                                    rhs=kraw, start=True, stop=True)
                                ksw = ptmp.tile([64, 512], DT, tag="ksw")
                                nc.scalar.copy(ksw, psw[0:64, :])
                                t0 = ptmp.tile([64, 512], DT, tag="t0")
                                t1 = ptmp.tile([64, 512], DT, tag="t1")
                                nc.vector.tensor_mul(
                                    t0, kraw, cost[0:64, :])
                                nc.vector.tensor_mul(
                                    t1, ksw, sint[0:64, :])
                                nc.vector.tensor_add(kT_sb[0:64, cols], t0, t1)
                                for t4 in range(4):
                                    pvt = ps_vt.tile([128, 64], DT, tag="vt")
                                    nc.tensor.transpose(
                                        pvt, vraw[:, 128 * t4:128 * t4 + 128],
                                        id_sb)
                                    TT = (c0 + 512 * sub) // 128 + t4
                                    nc.vector.tensor_copy(
                                        v_sb[:, TT, 0:64], pvt)

            # ----------- phase 2: attention + gather + out-proj -----------
            with tc.tile_pool(name="ps_att", bufs=2, space="PSUM") as ps_att, \
                 tc.tile_pool(name="ps_pv", bufs=1, space="PSUM") as ps_pv, \
                 tc.tile_pool(name="ps_out", bufs=2, space="PSUM") as ps_out, \
                 tc.tile_pool(name="ptp", bufs=3) as ptp, \
                 tc.tile_pool(name="normp", bufs=2) as normp, \
                 tc.tile_pool(name="ytfp", bufs=2) as ytfp, \
                 tc.tile_pool(name="osbp", bufs=2) as osbp:
                nc.sync.dma_start(
                    out=wo_sb, in_=woT.rearrange("(a p) o -> p a o", p=128))

                def attention_qb(qb):
                    b, p = qb // PQB, qb % PQB
                    qcols = slice(256 * qb, 256 * qb + 256)
                    qcols_hi = slice(256 * qb + 128, 256 * qb + 256)
                    nk = 2 * (p + 1)
                    po = ps_pv.tile([65, 4, 256], F32, tag="pv", name="po")
                    for kt in range(nk):
                        last = (kt == nk - 1)
                        kc = slice(b * Tt + 128 * kt, b * Tt + 128 * kt + 128)
                        ktg = b * (Tt // 128) + kt
                        sm = ps_att.tile([128, 4, 256], F32, tag="smega")
                        pt = ptp.tile([128, 4, 256], DT, tag="pt")
                        if not last:
                            for hp in range(2):
                                nc.tensor.matmul(
                                    sm[:, 2 * hp:2 * hp + 2, :],
                                    lhsT=kT_sb[:, kc],
                                    rhs=qT_sb[:, 2 * hp:2 * hp + 2, qcols],
                                    start=True, stop=True)
                            nc.scalar.activation(
                                pt, sm, mybir.ActivationFunctionType.Exp,
                                scale=0.125)
                            if kt == nk - 2:
                                for h in range(4):
                                    nc.vector.tensor_mul(
                                        pt[:, h, :], pt[:, h, :], m0_sb)
                            for hp in range(2):
                                nc.tensor.matmul(
                                    po[:, 2 * hp:2 * hp + 2, :],
                                    lhsT=v_sb[:, ktg, :],
                                    rhs=pt[:, 2 * hp:2 * hp + 2, :],
                                    start=(kt == 0), stop=(kt == nk - 2))
                        else:
                            # diagonal upper k-tile: only q-cols 128..256 live
                            for h in range(4):
                                nc.tensor.matmul(
                                    sm[:, h, 128:256],
                                    lhsT=kT_sb[:, kc],
                                    rhs=qT_sb[:, h, qcols_hi],
                                    start=True, stop=True)
                            nc.scalar.activation(
                                pt[:, :, 128:256], sm[:, :, 128:256],
                                mybir.ActivationFunctionType.Exp, scale=0.125)
                            for h in range(4):
                                nc.vector.tensor_mul(
                                    pt[:, h, 128:256], pt[:, h, 128:256],
                                    m0_sb[:, 0:128])
                            for h in range(4):
                                nc.tensor.matmul(
                                    po[:, h, 128:256],
                                    lhsT=v_sb[:, ktg, :],
                                    rhs=pt[:, h, 128:256],
                                    start=False, stop=True,
                                    skip_group_check=True)
                    ssb4 = normp.tile([1, 4, 256], F32, tag="ssb4")
                    nc.vector.tensor_copy(ssb4, po[64:65, :, :])
                    posb = normp.tile([64, 4, 256], F32, tag="posb")
                    nc.vector.tensor_copy(posb, po[0:64, :, :])
                    bca = normp.tile([64, 4, 256], F32, tag="bca")
                    nc.gpsimd.partition_broadcast(bca, ssb4)
                    rec = normp.tile([64, 4, 256], F32, tag="rec")
                    nc.vector.reciprocal_approx_fast(rec, bca)
                    nc.vector.tensor_mul(yloc[:, :, qcols], posb, rec)

                def gather(ch):
                    ccols = slice(512 * ch, 512 * ch + 512)
                    nc.sync.dma_start(
                        out=ytloc[ch].rearrange("(h d) t -> d h t", h=HPC),
                        in_=yloc[:, :, ccols])
                    nc.gpsimd.collective_compute(
                        "AllGather", mybir.AluOpType.bypass,
                        replica_groups=[list(range(NCORES))],
                        ins=[ytloc[ch]], outs=[ytful[ch]])

                def prefetch_ytf(ch):
                    ytf = ytfp.tile([128, 16, 512], DT, tag="ytf",
                                    name=f"ytf{ch}")
                    for dt_ in range(16):
                        nc.scalar.dma_start(
                            out=ytf[:, dt_, :],
                            in_=ytful[ch][128 * dt_:128 * dt_ + 128, :])
                    return ytf

                def outproj(ch, ytf):
                    for ot in range(2):
                        pout = ps_out.tile([128, 512], F32, tag="out",
                                           name="pout")
                        for dt_ in range(16):
                            nc.tensor.matmul(
                                pout,
                                lhsT=wo_sb[:, dt_, 128 * ot:128 * ot + 128],
                                rhs=ytf[:, dt_, :],
                                start=(dt_ == 0), stop=(dt_ == 15))
                        ot_sb = osbp.tile([128, 512], F32, tag="osb")
                        nc.vector.tensor_copy(ot_sb, pout)
                        nc.sync.dma_start(
                            out=outp[128 * ot:128 * ot + 128,
                                     512 * ch:512 * ch + 512],
                            in_=ot_sb)

                order = ([0, 3, 1, 6, 4, 7, 5, 2] if NCH == 8
                         else list(range(NCH)))
                pend = []  # [(ch, ytf)] not yet out-projected
                for i, ch in enumerate(order):
                    attention_qb(2 * ch)
                    if i >= 1:
                        pend.append((order[i - 1],
                                     prefetch_ytf(order[i - 1])))
                    attention_qb(2 * ch + 1)
                    gather(ch)
                    if len(pend) >= 4:
                        outproj(*pend.pop(0))
                pend.append((order[-1], prefetch_ytf(order[-1])))
                for ch, ytf in pend:
                    outproj(ch, ytf)

    nc.compile()
    return nc


def host_inputs(x, cos, sin, wq, wk, wv, wo, Tt=T):
    """Build the 8 per-core input maps from full fp32 inputs."""
    BT = B * Tt
    x = np.asarray(x, np.float32)[:, :Tt, :]
    xT = np.ascontiguousarray(x.reshape(BT, DIM).T).astype(BF)

    cos = np.asarray(cos, np.float32)[:Tt]
    sin = np.asarray(sin, np.float32)[:Tt]
    cos2 = np.empty((128, BT), np.float32)
    sin2 = np.empty((128, BT), np.float32)
    for d in range(128):
        j = (d % 64) // 2
        cos2[d] = np.tile(cos[:, j], B)
        sin2[d] = np.tile(sin[:, j] if d % 2 else -sin[:, j], B)

    pswp = np.zeros((128, 128), BF)
    for i in range(128):
        pswp[i, i ^ 1] = 1
    ident = np.eye(64, dtype=BF)
    ii = np.arange(128)[:, None]
    jj = np.arange(256)[None, :]
    mask0 = (jj >= ii).astype(BF)
    mask1 = (jj >= ii + 128).astype(BF)

    maps = []
    for c in range(NCORES):
        qs = slice(256 * c, 256 * c + 256)
        ks = slice(64 * c, 64 * c + 64)
        wkv = np.concatenate([wk[ks], wv[ks]], axis=0)
        maps.append({
            "xT": xT,
            "wqT": np.ascontiguousarray(wq[qs].T).astype(BF),
            "wkvT": np.ascontiguousarray(wkv.T).astype(BF),
            "woT": np.ascontiguousarray(wo[qs].T).astype(BF),
            "cos2": cos2.astype(BF), "sin2": sin2.astype(BF),
            "pswp": pswp, "ident": ident,
            "mask0": mask0, "mask1": mask1,
        })
    return maps


_NC_CACHE = {}


def _get_nc(Tt=T):
    if Tt not in _NC_CACHE:
        _NC_CACHE[Tt] = build_nc(Tt)
    return _NC_CACHE[Tt]


def kernel(x, cos, sin, wq, wk, wv, wo):
    global LAST_RESULTS
    nc = _get_nc(T)
    maps = host_inputs(x, cos, sin, wq, wk, wv, wo)
    res = run_bass_kernel_spmd(nc, maps, core_ids=list(range(NCORES)))
    LAST_RESULTS = res
    out = np.empty((B * T, DIM), np.float32)
    for c in range(NCORES):
        out[:, 256 * c:256 * c + 256] = res.results[c]["out"].T
    return out.reshape(B, T, DIM)


# revision 19
# speedup vs baseline: 1.0779x; 1.0779x over previous
"""GroupedQueryAttention on 8 Trainium2 NeuronCores (Bass/Tile).

Tensor-parallel over heads: core c owns q-heads 4c..4c+3 and kv-head c.
Per core: bf16 projections + on-chip interleaved RoPE (pair-swap via a
permutation matmul), causal attention per 256-row q-block (softmax without
max-subtraction; denominator via a ones-column in the PV matmul), then an
AllGather of y^T and a transposed out-projection producing the core's
256-column slice of the output (host re-transposes and concatenates).

Attention matmuls process head-pairs (N=512 moving operand) and share the
stationary k/v tiles; softmax exp runs on ScalarE in [128,4,256] batches.
"""
import os
import sys
import types

os.environ.setdefault("JAX_PLATFORMS", "cpu,axon")

import numpy as np
import ml_dtypes

BF = ml_dtypes.bfloat16

# Optional NTFF-profile hook injection (lets BASS_TRACE=1 capture exec_time).
try:
    import antenv.axon_hooks  # noqa: F401
except ImportError:
    try:
        _hm = types.ModuleType("antenv.axon_hooks")
        _hs = [None]
        _hm.set_axon_ntff_profile_hook = lambda h: _hs.__setitem__(0, h)
        _hm.get_axon_ntff_profile_hook = lambda: _hs[0]
        sys.modules["antenv.axon_hooks"] = _hm
        import antenv

        antenv.axon_hooks = _hm
        from trn_agent_boot.trn_boot import _ntff_profile_via_ctypes

        _hook = _ntff_profile_via_ctypes("/opt/axon/libaxon_pjrt.so")
        if _hook is not None:
            _hm.set_axon_ntff_profile_hook(_hook)
    except Exception:
        pass

import concourse.bass as bass
import concourse.tile as tile
from concourse import bacc, mybir
from concourse.bass_utils import run_bass_kernel_spmd

B, T, DIM = 2, 2048, 2048
N_HEADS, N_KV_HEADS, HEAD_DIM = 32, 8, 64
NCORES = 8
HPC = N_HEADS // NCORES  # 4 q heads per core
DT = mybir.dt.bfloat16
F32 = mybir.dt.float32

LAST_RESULTS = None  # BassKernelResults of the most recent run (for test.py)


def build_nc(Tt=T):
    """Build + compile the SPMD program (same for all 8 cores)."""
    BT = B * Tt
    PQB = Tt // 256  # q-blocks per batch
    NCH = BT // 512  # all-gather chunks
    NTT = BT // 128  # 128-token tiles
    NGG = BT // 512  # x streaming groups
    assert BT % 1024 == 0

    nc = bacc.Bacc("TRN2", target_bir_lowering=False, debug=False,
                   num_devices=NCORES)

    xT = nc.declare_dram_parameter("xT", [DIM, BT], DT, isOutput=False)
    wqT = nc.declare_dram_parameter("wqT", [DIM, 256], DT, isOutput=False)
    wkvT = nc.declare_dram_parameter("wkvT", [DIM, 128], DT, isOutput=False)
    woT = nc.declare_dram_parameter("woT", [DIM, 256], DT, isOutput=False)
    cos2 = nc.declare_dram_parameter("cos2", [128, BT], DT, isOutput=False)
    sin2 = nc.declare_dram_parameter("sin2", [128, BT], DT, isOutput=False)
    pswp = nc.declare_dram_parameter("pswp", [128, 128], DT, isOutput=False)
    ident = nc.declare_dram_parameter("ident", [64, 64], DT, isOutput=False)
    mask0 = nc.declare_dram_parameter("mask0", [128, 256], DT, isOutput=False)
    mask1 = nc.declare_dram_parameter("mask1", [128, 256], DT, isOutput=False)
    outp = nc.declare_dram_parameter("out", [256, BT], F32, isOutput=True)

    with tile.TileContext(nc) as tc:
        with tc.tile_pool(name="persist", bufs=1) as persist, \
             tc.tile_pool(name="dram", bufs=1, space="DRAM") as dramp:
            qT_sb = persist.tile([128, HPC, BT], DT)
            kT_sb = persist.tile([128, BT], DT)
            v_sb = persist.tile([128, NTT, 65], DT)
            yloc = persist.tile([64, HPC, BT], DT)
            wo_sb = persist.tile([128, 16, 256], DT)
            pswp_sb = persist.tile([128, 128], DT)
            id_sb = persist.tile([64, 64], DT)
            m0_sb = persist.tile([128, 256], DT)
            m1_sb = persist.tile([128, 256], DT)

            ytloc = [dramp.tile([256, 512], DT, name=f"ytloc{c}")
                     for c in range(NCH)]
            ytful = [dramp.tile([2048, 512], DT, addr_space="Shared",
                                name=f"ytful{c}") for c in range(NCH)]

            # ---------------- phase 1: projections + RoPE ----------------
            with tc.tile_pool(name="wpool", bufs=1) as wpool, \
                 tc.tile_pool(name="xs", bufs=2) as xs, \
                 tc.tile_pool(name="cs", bufs=2) as cs, \
                 tc.tile_pool(name="ptmp", bufs=3) as ptmp, \
                 tc.tile_pool(name="ps_mm", bufs=2, space="PSUM") as ps_mm, \
                 tc.tile_pool(name="ps_sw", bufs=2, space="PSUM") as ps_sw, \
                 tc.tile_pool(name="ps_vt", bufs=2, space="PSUM") as ps_vt:
                wq_sb = wpool.tile([128, 16, 256], DT)
                wkv_sb = wpool.tile([128, 16, 128], DT)
                for ct in range(16):
                    nc.sync.dma_start(out=wq_sb[:, ct, :],
                                      in_=wqT[128 * ct:128 * ct + 128, :])
                    nc.sync.dma_start(out=wkv_sb[:, ct, :],
                                      in_=wkvT[128 * ct:128 * ct + 128, :])
                nc.sync.dma_start(out=pswp_sb, in_=pswp[:])
                nc.sync.dma_start(out=id_sb, in_=ident[:])
                nc.sync.dma_start(out=m0_sb, in_=mask0[:])
                nc.sync.dma_start(out=m1_sb, in_=mask1[:])
                nc.vector.memset(v_sb[:, :, 64:65], 1.0)
                nc.vector.memset(qT_sb[64:128, :, :], 0.0)
                nc.vector.memset(kT_sb[64:128, :], 0.0)

                for gg in range(NGG):
                    c0 = 512 * gg
                    xt = xs.tile([128, 16, 512], DT, tag="xt")
                    for ct in range(16):
                        nc.gpsimd.dma_start(
                            out=xt[:, ct, :],
                            in_=xT[128 * ct:128 * ct + 128, c0:c0 + 512])
                    cost = cs.tile([128, 512], DT, tag="cost")
                    sint = cs.tile([128, 512], DT, tag="sint")
                    nc.sync.dma_start(out=cost, in_=cos2[:, c0:c0 + 512])
                    nc.sync.dma_start(out=sint, in_=sin2[:, c0:c0 + 512])

                    # m = 0,1: q head-pairs; m = 2: kv
                    for m in range(3):
                        pmm = ps_mm.tile([128, 512], F32, tag="mm")
                        for ct in range(16):
                            if m < 2:
                                w_ap = wq_sb[:, ct, 128 * m:128 * m + 128]
                            else:
                                w_ap = wkv_sb[:, ct, :]
                            nc.tensor.matmul(
                                pmm, lhsT=w_ap, rhs=xt[:, ct, :],
                                start=(ct == 0), stop=(ct == 15))
                        for sub in range(1):
                            cols = slice(c0, c0 + 512)
                            lc = slice(0, 512)
                            if m < 2:
                                qraw = ptmp.tile([128, 512], DT, tag="qraw")
                                nc.scalar.copy(qraw, pmm)
                                psw = ps_sw.tile([128, 512], F32, tag="sw")
                                nc.tensor.matmul(psw, lhsT=pswp_sb, rhs=qraw,
                                                 start=True, stop=True)
                                qsw = ptmp.tile([128, 512], DT, tag="qsw")
                                nc.scalar.copy(qsw, psw)
                                for hh in range(2):
                                    h = 2 * m + hh
                                    rows = slice(64 * hh, 64 * hh + 64)
                                    t0 = ptmp.tile([64, 512], DT, tag="t0")
                                    t1 = ptmp.tile([64, 512], DT, tag="t1")
                                    nc.vector.tensor_mul(
                                        t0, qraw[rows, :], cost[rows, :])
                                    nc.vector.tensor_mul(
                                        t1, qsw[rows, :], sint[rows, :])
                                    nc.vector.tensor_add(
                                        qT_sb[0:64, h, cols], t0, t1)
                            else:
                                kraw = ptmp.tile([64, 512], DT, tag="kraw")
                                vraw = ptmp.tile([64, 512], DT, tag="vraw")
                                nc.scalar.copy(kraw, pmm[0:64, :])
                                nc.scalar.copy(vraw, pmm[64:128, :])
                                psw = ps_sw.tile([128, 512], F32, tag="sw")
                                nc.tensor.matmul(
                                    psw[0:64, :], lhsT=pswp_sb[0:64, 0:64],
                                    rhs=kraw, start=True, stop=True)
                                ksw = ptmp.tile([64, 512], DT, tag="ksw")
                                nc.scalar.copy(ksw, psw[0:64, :])
                                t0 = ptmp.tile([64, 512], DT, tag="t0")
                                t1 = ptmp.tile([64, 512], DT, tag="t1")
                                nc.vector.tensor_mul(
                                    t0, kraw, cost[0:64, :])
                                nc.vector.tensor_mul(
                                    t1, ksw, sint[0:64, :])
                                nc.vector.tensor_add(kT_sb[0:64, cols], t0, t1)
                                for t4 in range(4):
                                    pvt = ps_vt.tile([128, 64], DT, tag="vt")
                                    nc.tensor.transpose(
                                        pvt, vraw[:, 128 * t4:128 * t4 + 128],
                                        id_sb)
                                    TT = (c0 + 512 * sub) // 128 + t4
                                    nc.vector.tensor_copy(
                                        v_sb[:, TT, 0:64], pvt)

            # ----------- phase 2: attention + gather + out-proj -----------
            with tc.tile_pool(name="ps_att", bufs=2, space="PSUM") as ps_att, \
                 tc.tile_pool(name="ps_pv", bufs=1, space="PSUM") as ps_pv, \
                 tc.tile_pool(name="ps_out", bufs=2, space="PSUM") as ps_out, \
                 tc.tile_pool(name="ptp", bufs=3) as ptp, \
                 tc.tile_pool(name="normp", bufs=2) as normp, \
                 tc.tile_pool(name="ytfp", bufs=2) as ytfp, \
                 tc.tile_pool(name="osbp", bufs=2) as osbp:
                nc.sync.dma_start(
                    out=wo_sb, in_=woT.rearrange("(a p) o -> p a o", p=128))

                def attention_qb(qb):
                    b, p = qb // PQB, qb % PQB
                    qcols = slice(256 * qb, 256 * qb + 256)
                    qcols_hi = slice(256 * qb + 128, 256 * qb + 256)
                    nk = 2 * (p + 1)
                    po = ps_pv.tile([65, 4, 256], F32, tag="pv", name="po")
                    for kt in range(nk):
                        last = (kt == nk - 1)
                        kc = slice(b * Tt + 128 * kt, b * Tt + 128 * kt + 128)
                        ktg = b * (Tt // 128) + kt
                        sm = ps_att.tile([128, 4, 256], F32, tag="smega")
                        pt = ptp.tile([128, 4, 256], DT, tag="pt")
                        if not last:
                            for hp in range(2):
                                nc.tensor.matmul(
                                    sm[:, 2 * hp:2 * hp + 2, :],
                                    lhsT=kT_sb[:, kc],
                                    rhs=qT_sb[:, 2 * hp:2 * hp + 2, qcols],
                                    start=True, stop=True)
                            nc.scalar.activation(
                                pt, sm, mybir.ActivationFunctionType.Exp,
                                scale=0.125)
                            if kt == nk - 2:
                                for h in range(4):
                                    nc.vector.tensor_mul(
                                        pt[:, h, :], pt[:, h, :], m0_sb)
                            for hp in range(2):
                                nc.tensor.matmul(
                                    po[:, 2 * hp:2 * hp + 2, :],
                                    lhsT=v_sb[:, ktg, :],
                                    rhs=pt[:, 2 * hp:2 * hp + 2, :],
                                    start=(kt == 0), stop=(kt == nk - 2))
                        else:
                            # diagonal upper k-tile: only q-cols 128..256 live
                            for h in range(4):
                                nc.tensor.matmul(
                                    sm[:, h, 128:256],
                                    lhsT=kT_sb[:, kc],
                                    rhs=qT_sb[:, h, qcols_hi],
                                    start=True, stop=True)
                            nc.scalar.activation(
                                pt[:, :, 128:256], sm[:, :, 128:256],
                                mybir.ActivationFunctionType.Exp, scale=0.125)
                            for h in range(4):
                                nc.vector.tensor_mul(
                                    pt[:, h, 128:256], pt[:, h, 128:256],
                                    m0_sb[:, 0:128])
                            for h in range(4):
                                nc.tensor.matmul(
                                    po[:, h, 128:256],
                                    lhsT=v_sb[:, ktg, :],
                                    rhs=pt[:, h, 128:256],
                                    start=False, stop=True,
                                    skip_group_check=True)
                    ssb4 = normp.tile([1, 4, 256], F32, tag="ssb4")
                    nc.vector.tensor_copy(ssb4, po[64:65, :, :])
                    posb = normp.tile([64, 4, 256], F32, tag="posb")
                    nc.vector.tensor_copy(posb, po[0:64, :, :])
                    bca = normp.tile([64, 4, 256], F32, tag="bca")
                    nc.gpsimd.partition_broadcast(bca, ssb4)
                    rec = normp.tile([64, 4, 256], F32, tag="rec")
                    nc.vector.reciprocal_approx_fast(rec, bca)
                    nc.vector.tensor_mul(yloc[:, :, qcols], posb, rec)

                def gather(ch):
                    ccols = slice(512 * ch, 512 * ch + 512)
                    nc.sync.dma_start(
                        out=ytloc[ch].rearrange("(h d) t -> d h t", h=HPC),
                        in_=yloc[:, :, ccols])
                    nc.gpsimd.collective_compute(
                        "AllGather", mybir.AluOpType.bypass,
                        replica_groups=[list(range(NCORES))],
                        ins=[ytloc[ch]], outs=[ytful[ch]])

                def outproj(ch):
                    ytf = ytfp.tile([128, 16, 512], DT, tag="ytf",
                                    name=f"ytf{ch}")
                    for dt_ in range(16):
                        nc.sync.dma_start(
                            out=ytf[:, dt_, :],
                            in_=ytful[ch][128 * dt_:128 * dt_ + 128, :])
                    for ot in range(2):
                        pout = ps_out.tile([128, 512], F32, tag="out",
                                           name="pout")
                        for dt_ in range(16):
                            nc.tensor.matmul(
                                pout,
                                lhsT=wo_sb[:, dt_, 128 * ot:128 * ot + 128],
                                rhs=ytf[:, dt_, :],
                                start=(dt_ == 0), stop=(dt_ == 15))
                        ot_sb = osbp.tile([128, 512], F32, tag="osb")
                        nc.vector.tensor_copy(ot_sb, pout)
                        nc.sync.dma_start(
                            out=outp[128 * ot:128 * ot + 128,
                                     512 * ch:512 * ch + 512],
                            in_=ot_sb)

                order = ([0, 3, 1, 6, 4, 7, 5, 2] if NCH == 8
                         else list(range(NCH)))
                pend = []  # chunks gathered but not yet out-projected
                for i, ch in enumerate(order):
                    attention_qb(2 * ch)
                    if len(pend) >= 3:
                        outproj(pend.pop(0))
                    attention_qb(2 * ch + 1)
                    gather(ch)
                    pend.append(ch)
                for ch in pend:
                    outproj(ch)

    nc.compile()
    return nc


def host_inputs(x, cos, sin, wq, wk, wv, wo, Tt=T):
    """Build the 8 per-core input maps from full fp32 inputs."""
    BT = B * Tt
    x = np.asarray(x, np.float32)[:, :Tt, :]
    xT = np.ascontiguousarray(x.reshape(BT, DIM).T).astype(BF)

    cos = np.asarray(cos, np.float32)[:Tt]
    sin = np.asarray(sin, np.float32)[:Tt]
    cos2 = np.empty((128, BT), np.float32)
    sin2 = np.empty((128, BT), np.float32)
    for d in range(128):
        j = (d % 64) // 2
        cos2[d] = np.tile(cos[:, j], B)
        sin2[d] = np.tile(sin[:, j] if d % 2 else -sin[:, j], B)

    pswp = np.zeros((128, 128), BF)
    for i in range(128):
        pswp[i, i ^ 1] = 1
    ident = np.eye(64, dtype=BF)
    ii = np.arange(128)[:, None]
    jj = np.arange(256)[None, :]
    mask0 = (jj >= ii).astype(BF)
    mask1 = (jj >= ii + 128).astype(BF)

    maps = []
    for c in range(NCORES):
        qs = slice(256 * c, 256 * c + 256)
        ks = slice(64 * c, 64 * c + 64)
        wkv = np.concatenate([wk[ks], wv[ks]], axis=0)
        maps.append({
            "xT": xT,
            "wqT": np.ascontiguousarray(wq[qs].T).astype(BF),
            "wkvT": np.ascontiguousarray(wkv.T).astype(BF),
            "woT": np.ascontiguousarray(wo[qs].T).astype(BF),
            "cos2": cos2.astype(BF), "sin2": sin2.astype(BF),
            "pswp": pswp, "ident": ident,
            "mask0": mask0, "mask1": mask1,
        })
    return maps


_NC_CACHE = {}


def _get_nc(Tt=T):
    if Tt not in _NC_CACHE:
        _NC_CACHE[Tt] = build_nc(Tt)
    return _NC_CACHE[Tt]


def kernel(x, cos, sin, wq, wk, wv, wo):
    global LAST_RESULTS
    nc = _get_nc(T)
    maps = host_inputs(x, cos, sin, wq, wk, wv, wo)
    res = run_bass_kernel_spmd(nc, maps, core_ids=list(range(NCORES)))
    LAST_RESULTS = res
    out = np.empty((B * T, DIM), np.float32)
    for c in range(NCORES):
        out[:, 256 * c:256 * c + 256] = res.results[c]["out"].T
    return out.reshape(B, T, DIM)


# revision 20
# speedup vs baseline: 1.1063x; 1.0263x over previous
"""GroupedQueryAttention on 8 Trainium2 NeuronCores (Bass/Tile).

Tensor-parallel over heads: core c owns q-heads 4c..4c+3 and kv-head c.
Per core: bf16 projections + on-chip interleaved RoPE (pair-swap via a
permutation matmul), causal attention per 256-row q-block (softmax without
max-subtraction; denominator via a ones-column in the PV matmul), then an
AllGather of y^T and a transposed out-projection producing the core's
256-column slice of the output (host re-transposes and concatenates).

Attention matmuls process head-pairs (N=512 moving operand) and share the
stationary k/v tiles; softmax exp runs on ScalarE in [128,4,256] batches.
"""
import os
import sys
import types

os.environ.setdefault("JAX_PLATFORMS", "cpu,axon")

import numpy as np
import ml_dtypes

BF = ml_dtypes.bfloat16

# Optional NTFF-profile hook injection (lets BASS_TRACE=1 capture exec_time).
try:
    import antenv.axon_hooks  # noqa: F401
except ImportError:
    try:
        _hm = types.ModuleType("antenv.axon_hooks")
        _hs = [None]
        _hm.set_axon_ntff_profile_hook = lambda h: _hs.__setitem__(0, h)
        _hm.get_axon_ntff_profile_hook = lambda: _hs[0]
        sys.modules["antenv.axon_hooks"] = _hm
        import antenv

        antenv.axon_hooks = _hm
        from trn_agent_boot.trn_boot import _ntff_profile_via_ctypes

        _hook = _ntff_profile_via_ctypes("/opt/axon/libaxon_pjrt.so")
        if _hook is not None:
            _hm.set_axon_ntff_profile_hook(_hook)
    except Exception:
        pass

import concourse.bass as bass
import concourse.tile as tile
from concourse import bacc, mybir
from concourse.bass_utils import run_bass_kernel_spmd

B, T, DIM = 2, 2048, 2048
N_HEADS, N_KV_HEADS, HEAD_DIM = 32, 8, 64
NCORES = 8
HPC = N_HEADS // NCORES  # 4 q heads per core
DT = mybir.dt.bfloat16
F32 = mybir.dt.float32

LAST_RESULTS = None  # BassKernelResults of the most recent run (for test.py)


def build_nc(Tt=T):
    """Build + compile the SPMD program (same for all 8 cores)."""
    BT = B * Tt
    PQB = Tt // 256  # q-blocks per batch
    NCH = BT // 512  # all-gather chunks
    NTT = BT // 128  # 128-token tiles
    NGG = BT // 512  # x streaming groups
    assert BT % 1024 == 0

    nc = bacc.Bacc("TRN2", target_bir_lowering=False, debug=False,
                   num_devices=NCORES)

    xT = nc.declare_dram_parameter("xT", [DIM, BT], DT, isOutput=False)
    wqT = nc.declare_dram_parameter("wqT", [DIM, 256], DT, isOutput=False)
    wkvT = nc.declare_dram_parameter("wkvT", [DIM, 128], DT, isOutput=False)
    woT = nc.declare_dram_parameter("woT", [DIM, 256], DT, isOutput=False)
    cos2 = nc.declare_dram_parameter("cos2", [128, BT], DT, isOutput=False)
    sin2 = nc.declare_dram_parameter("sin2", [128, BT], DT, isOutput=False)
    pswp = nc.declare_dram_parameter("pswp", [128, 128], DT, isOutput=False)
    ident = nc.declare_dram_parameter("ident", [64, 64], DT, isOutput=False)
    mask0 = nc.declare_dram_parameter("mask0", [128, 256], DT, isOutput=False)
    mask1 = nc.declare_dram_parameter("mask1", [128, 256], DT, isOutput=False)
    outp = nc.declare_dram_parameter("out", [256, BT], F32, isOutput=True)

    with tile.TileContext(nc) as tc:
        with tc.tile_pool(name="persist", bufs=1) as persist, \
             tc.tile_pool(name="dram", bufs=1, space="DRAM") as dramp:
            qT_sb = persist.tile([128, HPC, BT], DT)
            kT_sb = persist.tile([128, BT], DT)
            v_sb = persist.tile([128, NTT, 65], DT)
            yloc = persist.tile([64, HPC, BT], DT)
            wo_sb = persist.tile([128, 16, 256], DT)
            pswp_sb = persist.tile([128, 128], DT)
            id_sb = persist.tile([64, 64], DT)
            m0_sb = persist.tile([128, 256], DT)
            m1_sb = persist.tile([128, 256], DT)

            ytloc = [dramp.tile([256, 512], DT, name=f"ytloc{c}")
                     for c in range(NCH)]
            ytful = [dramp.tile([2048, 512], DT, addr_space="Shared",
                                name=f"ytful{c}") for c in range(NCH)]

            # ---------------- phase 1: projections + RoPE ----------------
            with tc.tile_pool(name="wpool", bufs=1) as wpool, \
                 tc.tile_pool(name="xs", bufs=2) as xs, \
                 tc.tile_pool(name="cs", bufs=2) as cs, \
                 tc.tile_pool(name="ptmp", bufs=3) as ptmp, \
                 tc.tile_pool(name="ps_mm", bufs=2, space="PSUM") as ps_mm, \
                 tc.tile_pool(name="ps_sw", bufs=2, space="PSUM") as ps_sw, \
                 tc.tile_pool(name="ps_vt", bufs=2, space="PSUM") as ps_vt:
                wq_sb = wpool.tile([128, 16, 256], DT)
                wkv_sb = wpool.tile([128, 16, 128], DT)
                for ct in range(16):
                    nc.sync.dma_start(out=wq_sb[:, ct, :],
                                      in_=wqT[128 * ct:128 * ct + 128, :])
                    nc.sync.dma_start(out=wkv_sb[:, ct, :],
                                      in_=wkvT[128 * ct:128 * ct + 128, :])
                nc.sync.dma_start(out=pswp_sb, in_=pswp[:])
                nc.sync.dma_start(out=id_sb, in_=ident[:])
                nc.sync.dma_start(out=m0_sb, in_=mask0[:])
                nc.sync.dma_start(out=m1_sb, in_=mask1[:])
                nc.vector.memset(v_sb[:, :, 64:65], 1.0)
                nc.vector.memset(qT_sb[64:128, :, :], 0.0)
                nc.vector.memset(kT_sb[64:128, :], 0.0)

                for gg in range(NGG):
                    c0 = 512 * gg
                    xt = xs.tile([128, 16, 512], DT, tag="xt")
                    for ct in range(16):
                        nc.gpsimd.dma_start(
                            out=xt[:, ct, :],
                            in_=xT[128 * ct:128 * ct + 128, c0:c0 + 512])
                    cost = cs.tile([128, 512], DT, tag="cost")
                    sint = cs.tile([128, 512], DT, tag="sint")
                    nc.sync.dma_start(out=cost, in_=cos2[:, c0:c0 + 512])
                    nc.sync.dma_start(out=sint, in_=sin2[:, c0:c0 + 512])

                    # m = 0,1: q head-pairs; m = 2: kv
                    for m in range(3):
                        pmm = ps_mm.tile([128, 512], F32, tag="mm")
                        for ct in range(16):
                            if m < 2:
                                w_ap = wq_sb[:, ct, 128 * m:128 * m + 128]
                            else:
                                w_ap = wkv_sb[:, ct, :]
                            nc.tensor.matmul(
                                pmm, lhsT=w_ap, rhs=xt[:, ct, :],
                                start=(ct == 0), stop=(ct == 15))
                        for sub in range(1):
                            cols = slice(c0, c0 + 512)
                            lc = slice(0, 512)
                            if m < 2:
                                qraw = ptmp.tile([128, 512], DT, tag="qraw")
                                nc.scalar.copy(qraw, pmm)
                                psw = ps_sw.tile([128, 512], F32, tag="sw")
                                nc.tensor.matmul(psw, lhsT=pswp_sb, rhs=qraw,
                                                 start=True, stop=True)
                                qsw = ptmp.tile([128, 512], DT, tag="qsw")
                                nc.scalar.copy(qsw, psw)
                                for hh in range(2):
                                    h = 2 * m + hh
                                    rows = slice(64 * hh, 64 * hh + 64)
                                    t0 = ptmp.tile([64, 512], DT, tag="t0")
                                    t1 = ptmp.tile([64, 512], DT, tag="t1")
                                    nc.vector.tensor_mul(
                                        t0, qraw[rows, :], cost[rows, :])
                                    nc.vector.tensor_mul(
                                        t1, qsw[rows, :], sint[rows, :])
                                    nc.vector.tensor_add(
                                        qT_sb[0:64, h, cols], t0, t1)
                            else:
                                kraw = ptmp.tile([64, 512], DT, tag="kraw")
                                vraw = ptmp.tile([64, 512], DT, tag="vraw")
                                nc.scalar.copy(kraw, pmm[0:64, :])
                                nc.scalar.copy(vraw, pmm[64:128, :])
                                psw = ps_sw.tile([128, 512], F32, tag="sw")
                                nc.tensor.matmul(
                                    psw[0:64, :], lhsT=pswp_sb[0:64, 0:64],
                                    rhs=kraw, start=True, stop=True)
                                ksw = ptmp.tile([64, 512], DT, tag="ksw")
                                nc.scalar.copy(ksw, psw[0:64, :])
                                t0 = ptmp.tile([64, 512], DT, tag="t0")
                                t1 = ptmp.tile([64, 512], DT, tag="t1")
                                nc.vector.tensor_mul(
                                    t0, kraw, cost[0:64, :])
                                nc.vector.tensor_mul(
                                    t1, ksw, sint[0:64, :])
                                nc.vector.tensor_add(kT_sb[0:64, cols], t0, t1)
                                for t4 in range(4):
                                    pvt = ps_vt.tile([128, 64], DT, tag="vt")
                                    nc.tensor.transpose(
                                        pvt, vraw[:, 128 * t4:128 * t4 + 128],
                                        id_sb)
                                    TT = (c0 + 512 * sub) // 128 + t4
                                    nc.vector.tensor_copy(
                                        v_sb[:, TT, 0:64], pvt)

            # ----------- phase 2: attention + gather + out-proj -----------
            with tc.tile_pool(name="ps_att", bufs=2, space="PSUM") as ps_att, \
                 tc.tile_pool(name="ps_pv", bufs=1, space="PSUM") as ps_pv, \
                 tc.tile_pool(name="ps_out", bufs=2, space="PSUM") as ps_out, \
                 tc.tile_pool(name="ptp", bufs=3) as ptp, \
                 tc.tile_pool(name="normp", bufs=2) as normp, \
                 tc.tile_pool(name="ytfp", bufs=2) as ytfp, \
                 tc.tile_pool(name="osbp", bufs=2) as osbp:
                nc.sync.dma_start(
                    out=wo_sb, in_=woT.rearrange("(a p) o -> p a o", p=128))

                def attention_qb(qb):
                    b, p = qb // PQB, qb % PQB
                    qcols = slice(256 * qb, 256 * qb + 256)
                    qcols_hi = slice(256 * qb + 128, 256 * qb + 256)
                    nk = 2 * (p + 1)
                    po = ps_pv.tile([65, 4, 256], F32, tag="pv", name="po")
                    for kt in range(nk):
                        last = (kt == nk - 1)
                        kc = slice(b * Tt + 128 * kt, b * Tt + 128 * kt + 128)
                        ktg = b * (Tt // 128) + kt
                        sm = ps_att.tile([128, 4, 256], F32, tag="smega")
                        pt = ptp.tile([128, 4, 256], DT, tag="pt")
                        if not last:
                            for hp in range(2):
                                nc.tensor.matmul(
                                    sm[:, 2 * hp:2 * hp + 2, :],
                                    lhsT=kT_sb[:, kc],
                                    rhs=qT_sb[:, 2 * hp:2 * hp + 2, qcols],
                                    start=True, stop=True)
                            nc.scalar.activation(
                                pt, sm, mybir.ActivationFunctionType.Exp,
                                scale=0.125)
                            if kt == nk - 2:
                                for h in range(4):
                                    nc.vector.tensor_mul(
                                        pt[:, h, :], pt[:, h, :], m0_sb)
                            for hp in range(2):
                                nc.tensor.matmul(
                                    po[:, 2 * hp:2 * hp + 2, :],
                                    lhsT=v_sb[:, ktg, :],
                                    rhs=pt[:, 2 * hp:2 * hp + 2, :],
                                    start=(kt == 0), stop=(kt == nk - 2))
                        else:
                            # diagonal upper k-tile: only q-cols 128..256 live
                            for h in range(4):
                                nc.tensor.matmul(
                                    sm[:, h, 128:256],
                                    lhsT=kT_sb[:, kc],
                                    rhs=qT_sb[:, h, qcols_hi],
                                    start=True, stop=True)
                            nc.scalar.activation(
                                pt[:, :, 128:256], sm[:, :, 128:256],
                                mybir.ActivationFunctionType.Exp, scale=0.125)
                            for h in range(4):
                                nc.vector.tensor_mul(
                                    pt[:, h, 128:256], pt[:, h, 128:256],
                                    m0_sb[:, 0:128])
                            for h in range(4):
                                nc.tensor.matmul(
                                    po[:, h, 128:256],
                                    lhsT=v_sb[:, ktg, :],
                                    rhs=pt[:, h, 128:256],
                                    start=False, stop=True,
                                    skip_group_check=True)
                    ssb4 = normp.tile([1, 4, 256], F32, tag="ssb4")
                    nc.vector.tensor_copy(ssb4, po[64:65, :, :])
                    posb = normp.tile([64, 4, 256], F32, tag="posb")
                    nc.vector.tensor_copy(posb, po[0:64, :, :])
                    bca = normp.tile([64, 4, 256], F32, tag="bca")
                    nc.gpsimd.partition_broadcast(bca, ssb4)
                    rec = normp.tile([64, 4, 256], F32, tag="rec")
                    nc.vector.reciprocal_approx_fast(rec, bca)
                    nc.vector.tensor_mul(yloc[:, :, qcols], posb, rec)

                def gather(ch):
                    ccols = slice(512 * ch, 512 * ch + 512)
                    nc.sync.dma_start(
                        out=ytloc[ch].rearrange("(h d) t -> d h t", h=HPC),
                        in_=yloc[:, :, ccols])
                    nc.gpsimd.collective_compute(
                        "AllGather", mybir.AluOpType.bypass,
                        replica_groups=[list(range(NCORES))],
                        ins=[ytloc[ch]], outs=[ytful[ch]])

                def outproj(ch):
                    ytf = ytfp.tile([128, 16, 512], DT, tag="ytf",
                                    name=f"ytf{ch}")
                    for dt_ in range(16):
                        nc.sync.dma_start(
                            out=ytf[:, dt_, :],
                            in_=ytful[ch][128 * dt_:128 * dt_ + 128, :])
                    for ot in range(2):
                        pout = ps_out.tile([128, 512], F32, tag="out",
                                           name="pout")
                        for dt_ in range(16):
                            nc.tensor.matmul(
                                pout,
                                lhsT=wo_sb[:, dt_, 128 * ot:128 * ot + 128],
                                rhs=ytf[:, dt_, :],
                                start=(dt_ == 0), stop=(dt_ == 15))
                        ot_sb = osbp.tile([128, 512], F32, tag="osb")
                        nc.vector.tensor_copy(ot_sb, pout)
                        nc.sync.dma_start(
                            out=outp[128 * ot:128 * ot + 128,
                                     512 * ch:512 * ch + 512],
                            in_=ot_sb)

                order = ([0, 3, 1, 6, 4, 7, 5, 2] if NCH == 8
                         else list(range(NCH)))
                pend = []  # chunks gathered but not yet out-projected
                for i, ch in enumerate(order):
                    attention_qb(2 * ch)
                    attention_qb(2 * ch + 1)
                    gather(ch)
                    pend.append(ch)
                    if len(pend) >= 4:
                        outproj(pend.pop(0))
                for ch in pend:
                    outproj(ch)

    nc.compile()
    return nc


def host_inputs(x, cos, sin, wq, wk, wv, wo, Tt=T):
    """Build the 8 per-core input maps from full fp32 inputs."""
    BT = B * Tt
    x = np.asarray(x, np.float32)[:, :Tt, :]
    xT = np.ascontiguousarray(x.reshape(BT, DIM).T).astype(BF)

    cos = np.asarray(cos, np.float32)[:Tt]
    sin = np.asarray(sin, np.float32)[:Tt]
    cos2 = np.empty((128, BT), np.float32)
    sin2 = np.empty((128, BT), np.float32)
    for d in range(128):
        j = (d % 64) // 2
        cos2[d] = np.tile(cos[:, j], B)
        sin2[d] = np.tile(sin[:, j] if d % 2 else -sin[:, j], B)

    pswp = np.zeros((128, 128), BF)
    for i in range(128):
        pswp[i, i ^ 1] = 1
    ident = np.eye(64, dtype=BF)
    ii = np.arange(128)[:, None]
    jj = np.arange(256)[None, :]
    mask0 = (jj >= ii).astype(BF)
    mask1 = (jj >= ii + 128).astype(BF)

    maps = []
    for c in range(NCORES):
        qs = slice(256 * c, 256 * c + 256)
        ks = slice(64 * c, 64 * c + 64)
        wkv = np.concatenate([wk[ks], wv[ks]], axis=0)
        maps.append({
            "xT": xT,
            "wqT": np.ascontiguousarray(wq[qs].T).astype(BF),
            "wkvT": np.ascontiguousarray(wkv.T).astype(BF),
            "woT": np.ascontiguousarray(wo[qs].T).astype(BF),
            "cos2": cos2.astype(BF), "sin2": sin2.astype(BF),
            "pswp": pswp, "ident": ident,
            "mask0": mask0, "mask1": mask1,
        })
    return maps


_NC_CACHE = {}


def _get_nc(Tt=T):
    if Tt not in _NC_CACHE:
        _NC_CACHE[Tt] = build_nc(Tt)
    return _NC_CACHE[Tt]


def kernel(x, cos, sin, wq, wk, wv, wo):
    global LAST_RESULTS
    nc = _get_nc(T)
    maps = host_inputs(x, cos, sin, wq, wk, wv, wo)
    res = run_bass_kernel_spmd(nc, maps, core_ids=list(range(NCORES)))
    LAST_RESULTS = res
    out = np.empty((B * T, DIM), np.float32)
    for c in range(NCORES):
        out[:, 256 * c:256 * c + 256] = res.results[c]["out"].T
    return out.reshape(B, T, DIM)


# revision 21
# speedup vs baseline: 1.1071x; 1.0007x over previous
"""GroupedQueryAttention on 8 Trainium2 NeuronCores (Bass/Tile).

Tensor-parallel over heads: core c owns q-heads 4c..4c+3 and kv-head c.
Per core: bf16 projections + on-chip interleaved RoPE (pair-swap via a
permutation matmul), causal attention per 256-row q-block (softmax without
max-subtraction; denominator via a ones-column in the PV matmul), then an
AllGather of y^T and a transposed out-projection producing the core's
256-column slice of the output (host re-transposes and concatenates).

Attention matmuls process head-pairs (N=512 moving operand) and share the
stationary k/v tiles; softmax exp runs on ScalarE in [128,4,256] batches.
"""
import os
import sys
import types

os.environ.setdefault("JAX_PLATFORMS", "cpu,axon")

import numpy as np
import ml_dtypes

BF = ml_dtypes.bfloat16

# Optional NTFF-profile hook injection (lets BASS_TRACE=1 capture exec_time).
try:
    import antenv.axon_hooks  # noqa: F401
except ImportError:
    try:
        _hm = types.ModuleType("antenv.axon_hooks")
        _hs = [None]
        _hm.set_axon_ntff_profile_hook = lambda h: _hs.__setitem__(0, h)
        _hm.get_axon_ntff_profile_hook = lambda: _hs[0]
        sys.modules["antenv.axon_hooks"] = _hm
        import antenv

        antenv.axon_hooks = _hm
        from trn_agent_boot.trn_boot import _ntff_profile_via_ctypes

        _hook = _ntff_profile_via_ctypes("/opt/axon/libaxon_pjrt.so")
        if _hook is not None:
            _hm.set_axon_ntff_profile_hook(_hook)
    except Exception:
        pass

import concourse.bass as bass
import concourse.tile as tile
from concourse import bacc, mybir
from concourse.bass_utils import run_bass_kernel_spmd

B, T, DIM = 2, 2048, 2048
N_HEADS, N_KV_HEADS, HEAD_DIM = 32, 8, 64
NCORES = 8
HPC = N_HEADS // NCORES  # 4 q heads per core
DT = mybir.dt.bfloat16
F32 = mybir.dt.float32

LAST_RESULTS = None  # BassKernelResults of the most recent run (for test.py)


def build_nc(Tt=T):
    """Build + compile the SPMD program (same for all 8 cores)."""
    BT = B * Tt
    PQB = Tt // 256  # q-blocks per batch
    NCH = BT // 512  # all-gather chunks
    NTT = BT // 128  # 128-token tiles
    NGG = BT // 512  # x streaming groups
    assert BT % 1024 == 0

    nc = bacc.Bacc("TRN2", target_bir_lowering=False, debug=False,
                   num_devices=NCORES)

    xT = nc.declare_dram_parameter("xT", [DIM, BT], DT, isOutput=False)
    wqT = nc.declare_dram_parameter("wqT", [DIM, 256], DT, isOutput=False)
    wkvT = nc.declare_dram_parameter("wkvT", [DIM, 128], DT, isOutput=False)
    woT = nc.declare_dram_parameter("woT", [DIM, 256], DT, isOutput=False)
    cos2 = nc.declare_dram_parameter("cos2", [128, BT], DT, isOutput=False)
    sin2 = nc.declare_dram_parameter("sin2", [128, BT], DT, isOutput=False)
    pswp = nc.declare_dram_parameter("pswp", [128, 128], DT, isOutput=False)
    ident = nc.declare_dram_parameter("ident", [64, 64], DT, isOutput=False)
    mask0 = nc.declare_dram_parameter("mask0", [128, 256], DT, isOutput=False)
    mask1 = nc.declare_dram_parameter("mask1", [128, 256], DT, isOutput=False)
    outp = nc.declare_dram_parameter("out", [256, BT], F32, isOutput=True)

    with tile.TileContext(nc) as tc:
        with tc.tile_pool(name="persist", bufs=1) as persist, \
             tc.tile_pool(name="dram", bufs=1, space="DRAM") as dramp:
            qT_sb = persist.tile([128, HPC, BT], DT)
            kT_sb = persist.tile([128, BT], DT)
            v_sb = persist.tile([128, NTT, 65], DT)
            yloc = persist.tile([64, HPC, BT], DT)
            wo_sb = persist.tile([128, 16, 256], DT)
            pswp_sb = persist.tile([128, 128], DT)
            id_sb = persist.tile([64, 64], DT)
            m0_sb = persist.tile([128, 256], DT)
            m1_sb = persist.tile([128, 256], DT)

            ytloc = [dramp.tile([256, 512], DT, name=f"ytloc{c}")
                     for c in range(NCH)]
            ytful = [dramp.tile([2048, 512], DT, addr_space="Shared",
                                name=f"ytful{c}") for c in range(NCH)]

            # ---------------- phase 1: projections + RoPE ----------------
            with tc.tile_pool(name="wpool", bufs=1) as wpool, \
                 tc.tile_pool(name="xs", bufs=2) as xs, \
                 tc.tile_pool(name="cs", bufs=2) as cs, \
                 tc.tile_pool(name="ptmp", bufs=3) as ptmp, \
                 tc.tile_pool(name="ps_mm", bufs=2, space="PSUM") as ps_mm, \
                 tc.tile_pool(name="ps_sw", bufs=2, space="PSUM") as ps_sw, \
                 tc.tile_pool(name="ps_vt", bufs=2, space="PSUM") as ps_vt:
                wq_sb = wpool.tile([128, 16, 256], DT)
                wkv_sb = wpool.tile([128, 16, 128], DT)
                for ct in range(16):
                    nc.sync.dma_start(out=wq_sb[:, ct, :],
                                      in_=wqT[128 * ct:128 * ct + 128, :])
                    nc.sync.dma_start(out=wkv_sb[:, ct, :],
                                      in_=wkvT[128 * ct:128 * ct + 128, :])
                nc.sync.dma_start(out=pswp_sb, in_=pswp[:])
                nc.sync.dma_start(out=id_sb, in_=ident[:])
                nc.sync.dma_start(out=m0_sb, in_=mask0[:])
                nc.sync.dma_start(out=m1_sb, in_=mask1[:])
                nc.vector.memset(v_sb[:, :, 64:65], 1.0)
                nc.vector.memset(qT_sb[64:128, :, :], 0.0)
                nc.vector.memset(kT_sb[64:128, :], 0.0)

                for gg in range(NGG):
                    c0 = 512 * gg
                    xt = xs.tile([128, 16, 512], DT, tag="xt")
                    for ct in range(16):
                        nc.gpsimd.dma_start(
                            out=xt[:, ct, :],
                            in_=xT[128 * ct:128 * ct + 128, c0:c0 + 512])
                    cost = cs.tile([128, 512], DT, tag="cost")
                    sint = cs.tile([128, 512], DT, tag="sint")
                    nc.sync.dma_start(out=cost, in_=cos2[:, c0:c0 + 512])
                    nc.sync.dma_start(out=sint, in_=sin2[:, c0:c0 + 512])

                    # m = 0,1: q head-pairs; m = 2: kv
                    for m in range(3):
                        pmm = ps_mm.tile([128, 512], F32, tag="mm")
                        for ct in range(16):
                            if m < 2:
                                w_ap = wq_sb[:, ct, 128 * m:128 * m + 128]
                            else:
                                w_ap = wkv_sb[:, ct, :]
                            nc.tensor.matmul(
                                pmm, lhsT=w_ap, rhs=xt[:, ct, :],
                                start=(ct == 0), stop=(ct == 15))
                        for sub in range(1):
                            cols = slice(c0, c0 + 512)
                            lc = slice(0, 512)
                            if m < 2:
                                qraw = ptmp.tile([128, 512], DT, tag="qraw")
                                nc.scalar.copy(qraw, pmm)
                                psw = ps_sw.tile([128, 512], F32, tag="sw")
                                nc.tensor.matmul(psw, lhsT=pswp_sb, rhs=qraw,
                                                 start=True, stop=True)
                                qsw = ptmp.tile([128, 512], DT, tag="qsw")
                                nc.scalar.copy(qsw, psw)
                                for hh in range(2):
                                    h = 2 * m + hh
                                    rows = slice(64 * hh, 64 * hh + 64)
                                    t0 = ptmp.tile([64, 512], DT, tag="t0")
                                    t1 = ptmp.tile([64, 512], DT, tag="t1")
                                    nc.vector.tensor_mul(
                                        t0, qraw[rows, :], cost[rows, :])
                                    nc.vector.tensor_mul(
                                        t1, qsw[rows, :], sint[rows, :])
                                    nc.vector.tensor_add(
                                        qT_sb[0:64, h, cols], t0, t1)
                            else:
                                kraw = ptmp.tile([64, 512], DT, tag="kraw")
                                vraw = ptmp.tile([64, 512], DT, tag="vraw")
                                nc.scalar.copy(kraw, pmm[0:64, :])
                                nc.scalar.copy(vraw, pmm[64:128, :])
                                psw = ps_sw.tile([128, 512], F32, tag="sw")
                                nc.tensor.matmul(
                                    psw[0:64, :], lhsT=pswp_sb[0:64, 0:64],
                                    rhs=kraw, start=True, stop=True)
                                ksw = ptmp.tile([64, 512], DT, tag="ksw")
                                nc.scalar.copy(ksw, psw[0:64, :])
                                t0 = ptmp.tile([64, 512], DT, tag="t0")
                                t1 = ptmp.tile([64, 512], DT, tag="t1")
                                nc.vector.tensor_mul(
                                    t0, kraw, cost[0:64, :])
                                nc.vector.tensor_mul(
                                    t1, ksw, sint[0:64, :])
                                nc.vector.tensor_add(kT_sb[0:64, cols], t0, t1)
                                for t4 in range(4):
                                    pvt = ps_vt.tile([128, 64], DT, tag="vt")
                                    nc.tensor.transpose(
                                        pvt, vraw[:, 128 * t4:128 * t4 + 128],
                                        id_sb)
                                    TT = (c0 + 512 * sub) // 128 + t4
                                    nc.vector.tensor_copy(
                                        v_sb[:, TT, 0:64], pvt)

            # ----------- phase 2: attention + gather + out-proj -----------
            with tc.tile_pool(name="ps_att", bufs=2, space="PSUM") as ps_att, \
                 tc.tile_pool(name="ps_pv", bufs=1, space="PSUM") as ps_pv, \
                 tc.tile_pool(name="ps_out", bufs=2, space="PSUM") as ps_out, \
                 tc.tile_pool(name="ptp", bufs=4) as ptp, \
                 tc.tile_pool(name="normp", bufs=2) as normp, \
                 tc.tile_pool(name="ytfp", bufs=3) as ytfp, \
                 tc.tile_pool(name="osbp", bufs=2) as osbp:
                nc.sync.dma_start(
                    out=wo_sb, in_=woT.rearrange("(a p) o -> p a o", p=128))

                def attention_qb(qb):
                    b, p = qb // PQB, qb % PQB
                    qcols = slice(256 * qb, 256 * qb + 256)
                    qcols_hi = slice(256 * qb + 128, 256 * qb + 256)
                    nk = 2 * (p + 1)
                    po = ps_pv.tile([65, 4, 256], F32, tag="pv", name="po")
                    for kt in range(nk):
                        last = (kt == nk - 1)
                        kc = slice(b * Tt + 128 * kt, b * Tt + 128 * kt + 128)
                        ktg = b * (Tt // 128) + kt
                        sm = ps_att.tile([128, 4, 256], F32, tag="smega")
                        pt = ptp.tile([128, 4, 256], DT, tag="pt")
                        if not last:
                            for hp in range(2):
                                nc.tensor.matmul(
                                    sm[:, 2 * hp:2 * hp + 2, :],
                                    lhsT=kT_sb[:, kc],
                                    rhs=qT_sb[:, 2 * hp:2 * hp + 2, qcols],
                                    start=True, stop=True)
                            nc.scalar.activation(
                                pt, sm, mybir.ActivationFunctionType.Exp,
                                scale=0.125)
                            if kt == nk - 2:
                                for h in range(4):
                                    nc.vector.tensor_mul(
                                        pt[:, h, :], pt[:, h, :], m0_sb)
                            for hp in range(2):
                                nc.tensor.matmul(
                                    po[:, 2 * hp:2 * hp + 2, :],
                                    lhsT=v_sb[:, ktg, :],
                                    rhs=pt[:, 2 * hp:2 * hp + 2, :],
                                    start=(kt == 0), stop=(kt == nk - 2))
                        else:
                            # diagonal upper k-tile: only q-cols 128..256 live
                            for h in range(4):
                                nc.tensor.matmul(
                                    sm[:, h, 128:256],
                                    lhsT=kT_sb[:, kc],
                                    rhs=qT_sb[:, h, qcols_hi],
                                    start=True, stop=True)
                            nc.scalar.activation(
                                pt[:, :, 128:256], sm[:, :, 128:256],
                                mybir.ActivationFunctionType.Exp, scale=0.125)
                            for h in range(4):
                                nc.vector.tensor_mul(
                                    pt[:, h, 128:256], pt[:, h, 128:256],
                                    m0_sb[:, 0:128])
                            for h in range(4):
                                nc.tensor.matmul(
                                    po[:, h, 128:256],
                                    lhsT=v_sb[:, ktg, :],
                                    rhs=pt[:, h, 128:256],
                                    start=False, stop=True,
                                    skip_group_check=True)
                    ssb4 = normp.tile([1, 4, 256], F32, tag="ssb4")
                    nc.vector.tensor_copy(ssb4, po[64:65, :, :])
                    posb = normp.tile([64, 4, 256], F32, tag="posb")
                    nc.vector.tensor_copy(posb, po[0:64, :, :])
                    bca = normp.tile([64, 4, 256], F32, tag="bca")
                    nc.gpsimd.partition_broadcast(bca, ssb4)
                    rec = normp.tile([64, 4, 256], F32, tag="rec")
                    nc.vector.reciprocal_approx_fast(rec, bca)
                    nc.vector.tensor_mul(yloc[:, :, qcols], posb, rec)

                def gather(ch):
                    ccols = slice(512 * ch, 512 * ch + 512)
                    nc.sync.dma_start(
                        out=ytloc[ch].rearrange("(h d) t -> d h t", h=HPC),
                        in_=yloc[:, :, ccols])
                    nc.gpsimd.collective_compute(
                        "AllGather", mybir.AluOpType.bypass,
                        replica_groups=[list(range(NCORES))],
                        ins=[ytloc[ch]], outs=[ytful[ch]])

                def outproj(ch):
                    ytf = ytfp.tile([128, 16, 512], DT, tag="ytf",
                                    name=f"ytf{ch}")
                    for dt_ in range(16):
                        nc.sync.dma_start(
                            out=ytf[:, dt_, :],
                            in_=ytful[ch][128 * dt_:128 * dt_ + 128, :])
                    for ot in range(2):
                        pout = ps_out.tile([128, 512], F32, tag="out",
                                           name="pout")
                        for dt_ in range(16):
                            nc.tensor.matmul(
                                pout,
                                lhsT=wo_sb[:, dt_, 128 * ot:128 * ot + 128],
                                rhs=ytf[:, dt_, :],
                                start=(dt_ == 0), stop=(dt_ == 15))
                        ot_sb = osbp.tile([128, 512], F32, tag="osb")
                        nc.vector.tensor_copy(ot_sb, pout)
                        nc.sync.dma_start(
                            out=outp[128 * ot:128 * ot + 128,
                                     512 * ch:512 * ch + 512],
                            in_=ot_sb)

                order = ([0, 3, 1, 6, 4, 7, 5, 2] if NCH == 8
                         else list(range(NCH)))
                pend = []  # chunks gathered but not yet out-projected
                for i, ch in enumerate(order):
                    attention_qb(2 * ch)
                    attention_qb(2 * ch + 1)
                    gather(ch)
                    pend.append(ch)
                    if len(pend) >= 4:
                        outproj(pend.pop(0))
                for ch in pend:
                    outproj(ch)

    nc.compile()
    return nc


def host_inputs(x, cos, sin, wq, wk, wv, wo, Tt=T):
    """Build the 8 per-core input maps from full fp32 inputs."""
    BT = B * Tt
    x = np.asarray(x, np.float32)[:, :Tt, :]
    xT = np.ascontiguousarray(x.reshape(BT, DIM).T).astype(BF)

    cos = np.asarray(cos, np.float32)[:Tt]
    sin = np.asarray(sin, np.float32)[:Tt]
    cos2 = np.empty((128, BT), np.float32)
    sin2 = np.empty((128, BT), np.float32)
    for d in range(128):
        j = (d % 64) // 2
        cos2[d] = np.tile(cos[:, j], B)
        sin2[d] = np.tile(sin[:, j] if d % 2 else -sin[:, j], B)

    pswp = np.zeros((128, 128), BF)
    for i in range(128):
        pswp[i, i ^ 1] = 1
    ident = np.eye(64, dtype=BF)
    ii = np.arange(128)[:, None]
    jj = np.arange(256)[None, :]
    mask0 = (jj >= ii).astype(BF)
    mask1 = (jj >= ii + 128).astype(BF)

    maps = []
    for c in range(NCORES):
        qs = slice(256 * c, 256 * c + 256)
        ks = slice(64 * c, 64 * c + 64)
        wkv = np.concatenate([wk[ks], wv[ks]], axis=0)
        maps.append({
            "xT": xT,
            "wqT": np.ascontiguousarray(wq[qs].T).astype(BF),
            "wkvT": np.ascontiguousarray(wkv.T).astype(BF),
            "woT": np.ascontiguousarray(wo[qs].T).astype(BF),
            "cos2": cos2.astype(BF), "sin2": sin2.astype(BF),
            "pswp": pswp, "ident": ident,
            "mask0": mask0, "mask1": mask1,
        })
    return maps


_NC_CACHE = {}


def _get_nc(Tt=T):
    if Tt not in _NC_CACHE:
        _NC_CACHE[Tt] = build_nc(Tt)
    return _NC_CACHE[Tt]


def kernel(x, cos, sin, wq, wk, wv, wo):
    global LAST_RESULTS
    nc = _get_nc(T)
    maps = host_inputs(x, cos, sin, wq, wk, wv, wo)
    res = run_bass_kernel_spmd(nc, maps, core_ids=list(range(NCORES)))
    LAST_RESULTS = res
    out = np.empty((B * T, DIM), np.float32)
    for c in range(NCORES):
        out[:, 256 * c:256 * c + 256] = res.results[c]["out"].T
    return out.reshape(B, T, DIM)


# revision 22
# speedup vs baseline: 1.1438x; 1.0331x over previous
"""GroupedQueryAttention on 8 Trainium2 NeuronCores (Bass/Tile).

Tensor-parallel over heads: core c owns q-heads 4c..4c+3 and kv-head c.
Per core: bf16 projections + on-chip interleaved RoPE (pair-swap via a
permutation matmul), causal attention per 256-row q-block (softmax without
max-subtraction; denominator via a ones-column in the PV matmul), then an
AllGather of y^T and a transposed out-projection producing the core's
256-column slice of the output (host re-transposes and concatenates).

Attention matmuls process head-pairs (N=512 moving operand) and share the
stationary k/v tiles; softmax exp runs on ScalarE in [128,4,256] batches.
"""
import os
import sys
import types

os.environ.setdefault("JAX_PLATFORMS", "cpu,axon")

import numpy as np
import ml_dtypes

BF = ml_dtypes.bfloat16

# Optional NTFF-profile hook injection (lets BASS_TRACE=1 capture exec_time).
try:
    import antenv.axon_hooks  # noqa: F401
except ImportError:
    try:
        _hm = types.ModuleType("antenv.axon_hooks")
        _hs = [None]
        _hm.set_axon_ntff_profile_hook = lambda h: _hs.__setitem__(0, h)
        _hm.get_axon_ntff_profile_hook = lambda: _hs[0]
        sys.modules["antenv.axon_hooks"] = _hm
        import antenv

        antenv.axon_hooks = _hm
        from trn_agent_boot.trn_boot import _ntff_profile_via_ctypes

        _hook = _ntff_profile_via_ctypes("/opt/axon/libaxon_pjrt.so")
        if _hook is not None:
            _hm.set_axon_ntff_profile_hook(_hook)
    except Exception:
        pass

import concourse.bass as bass
import concourse.tile as tile
from concourse import bacc, mybir
from concourse.bass_utils import run_bass_kernel_spmd

B, T, DIM = 2, 2048, 2048
N_HEADS, N_KV_HEADS, HEAD_DIM = 32, 8, 64
NCORES = 8
HPC = N_HEADS // NCORES  # 4 q heads per core
DT = mybir.dt.bfloat16
F32 = mybir.dt.float32

LAST_RESULTS = None  # BassKernelResults of the most recent run (for test.py)


def build_nc(Tt=T):
    """Build + compile the SPMD program (same for all 8 cores)."""
    BT = B * Tt
    PQB = Tt // 256  # q-blocks per batch
    NCH = BT // 512  # all-gather chunks
    NTT = BT // 128  # 128-token tiles
    NGG = BT // 512  # x streaming groups
    assert BT % 1024 == 0

    nc = bacc.Bacc("TRN2", target_bir_lowering=False, debug=False,
                   num_devices=NCORES)

    xT = nc.declare_dram_parameter("xT", [DIM, BT], DT, isOutput=False)
    wqT = nc.declare_dram_parameter("wqT", [DIM, 256], DT, isOutput=False)
    wkvT = nc.declare_dram_parameter("wkvT", [DIM, 128], DT, isOutput=False)
    woT = nc.declare_dram_parameter("woT", [DIM, 256], DT, isOutput=False)
    cos2 = nc.declare_dram_parameter("cos2", [128, BT], DT, isOutput=False)
    sin2 = nc.declare_dram_parameter("sin2", [128, BT], DT, isOutput=False)
    pswp = nc.declare_dram_parameter("pswp", [128, 128], DT, isOutput=False)
    ident = nc.declare_dram_parameter("ident", [64, 64], DT, isOutput=False)
    mask0 = nc.declare_dram_parameter("mask0", [128, 256], DT, isOutput=False)
    mask1 = nc.declare_dram_parameter("mask1", [128, 256], DT, isOutput=False)
    outp = nc.declare_dram_parameter("out", [256, BT], F32, isOutput=True)

    with tile.TileContext(nc) as tc:
        with tc.tile_pool(name="persist", bufs=1) as persist, \
             tc.tile_pool(name="dram", bufs=1, space="DRAM") as dramp:
            qT_sb = persist.tile([128, HPC, BT], DT)
            kT_sb = persist.tile([128, BT], DT)
            v_sb = persist.tile([128, NTT, 65], DT)
            yloc = persist.tile([64, HPC, BT], DT)
            wo_sb = persist.tile([128, 16, 256], DT)
            pswp_sb = persist.tile([128, 128], DT)
            id_sb = persist.tile([64, 64], DT)
            m0_sb = persist.tile([128, 256], DT)
            m1_sb = persist.tile([128, 256], DT)

            ytloc = [dramp.tile([256, 512], DT, name=f"ytloc{c}")
                     for c in range(NCH)]
            ytful = [dramp.tile([2048, 512], DT, addr_space="Shared",
                                name=f"ytful{c}") for c in range(NCH)]

            # ---------------- phase 1: projections + RoPE ----------------
            with tc.tile_pool(name="wpool", bufs=1) as wpool, \
                 tc.tile_pool(name="xs", bufs=2) as xs, \
                 tc.tile_pool(name="cs", bufs=2) as cs, \
                 tc.tile_pool(name="ptmp", bufs=3) as ptmp, \
                 tc.tile_pool(name="ps_mm", bufs=2, space="PSUM") as ps_mm, \
                 tc.tile_pool(name="ps_sw", bufs=2, space="PSUM") as ps_sw, \
                 tc.tile_pool(name="ps_vt", bufs=2, space="PSUM") as ps_vt:
                wq_sb = wpool.tile([128, 16, 256], DT)
                wkv_sb = wpool.tile([128, 16, 128], DT)
                for ct in range(16):
                    nc.sync.dma_start(out=wq_sb[:, ct, :],
                                      in_=wqT[128 * ct:128 * ct + 128, :])
                    nc.sync.dma_start(out=wkv_sb[:, ct, :],
                                      in_=wkvT[128 * ct:128 * ct + 128, :])
                nc.sync.dma_start(out=pswp_sb, in_=pswp[:])
                nc.sync.dma_start(out=id_sb, in_=ident[:])
                nc.sync.dma_start(out=m0_sb, in_=mask0[:])
                nc.sync.dma_start(out=m1_sb, in_=mask1[:])
                nc.vector.memset(v_sb[:, :, 64:65], 1.0)
                nc.vector.memset(qT_sb[64:128, :, :], 0.0)
                nc.vector.memset(kT_sb[64:128, :], 0.0)

                for gg in range(NGG):
                    c0 = 512 * gg
                    xt = xs.tile([128, 16, 512], DT, tag="xt")
                    for ct in range(16):
                        nc.gpsimd.dma_start(
                            out=xt[:, ct, :],
                            in_=xT[128 * ct:128 * ct + 128, c0:c0 + 512])
                    cost = cs.tile([128, 512], DT, tag="cost")
                    sint = cs.tile([128, 512], DT, tag="sint")
                    nc.sync.dma_start(out=cost, in_=cos2[:, c0:c0 + 512])
                    nc.sync.dma_start(out=sint, in_=sin2[:, c0:c0 + 512])

                    # m = 0,1: q head-pairs; m = 2: kv
                    for m in range(3):
                        pmm = ps_mm.tile([128, 512], F32, tag="mm")
                        for ct in range(16):
                            if m < 2:
                                w_ap = wq_sb[:, ct, 128 * m:128 * m + 128]
                            else:
                                w_ap = wkv_sb[:, ct, :]
                            nc.tensor.matmul(
                                pmm, lhsT=w_ap, rhs=xt[:, ct, :],
                                start=(ct == 0), stop=(ct == 15))
                        for sub in range(1):
                            cols = slice(c0, c0 + 512)
                            lc = slice(0, 512)
                            if m < 2:
                                qraw = ptmp.tile([128, 512], DT, tag="qraw")
                                nc.scalar.copy(qraw, pmm)
                                psw = ps_sw.tile([128, 512], F32, tag="sw")
                                nc.tensor.matmul(psw, lhsT=pswp_sb, rhs=qraw,
                                                 start=True, stop=True)
                                qsw = ptmp.tile([128, 512], DT, tag="qsw")
                                nc.scalar.copy(qsw, psw)
                                for hh in range(2):
                                    h = 2 * m + hh
                                    rows = slice(64 * hh, 64 * hh + 64)
                                    t0 = ptmp.tile([64, 512], DT, tag="t0")
                                    t1 = ptmp.tile([64, 512], DT, tag="t1")
                                    nc.vector.tensor_mul(
                                        t0, qraw[rows, :], cost[rows, :])
                                    nc.vector.tensor_mul(
                                        t1, qsw[rows, :], sint[rows, :])
                                    nc.vector.tensor_add(
                                        qT_sb[0:64, h, cols], t0, t1)
                            else:
                                kraw = ptmp.tile([64, 512], DT, tag="kraw")
                                vraw = ptmp.tile([64, 512], DT, tag="vraw")
                                nc.scalar.copy(kraw, pmm[0:64, :])
                                nc.scalar.copy(vraw, pmm[64:128, :])
                                psw = ps_sw.tile([128, 512], F32, tag="sw")
                                nc.tensor.matmul(
                                    psw[0:64, :], lhsT=pswp_sb[0:64, 0:64],
                                    rhs=kraw, start=True, stop=True)
                                ksw = ptmp.tile([64, 512], DT, tag="ksw")
                                nc.scalar.copy(ksw, psw[0:64, :])
                                t0 = ptmp.tile([64, 512], DT, tag="t0")
                                t1 = ptmp.tile([64, 512], DT, tag="t1")
                                nc.vector.tensor_mul(
                                    t0, kraw, cost[0:64, :])
                                nc.vector.tensor_mul(
                                    t1, ksw, sint[0:64, :])
                                nc.vector.tensor_add(kT_sb[0:64, cols], t0, t1)
                                for t4 in range(4):
                                    pvt = ps_vt.tile([128, 64], DT, tag="vt")
                                    nc.tensor.transpose(
                                        pvt, vraw[:, 128 * t4:128 * t4 + 128],
                                        id_sb)
                                    TT = (c0 + 512 * sub) // 128 + t4
                                    nc.vector.tensor_copy(
                                        v_sb[:, TT, 0:64], pvt)

            # ----------- phase 2: attention + gather + out-proj -----------
            with tc.tile_pool(name="ps_att", bufs=2, space="PSUM") as ps_att, \
                 tc.tile_pool(name="ps_pv", bufs=1, space="PSUM") as ps_pv, \
                 tc.tile_pool(name="ps_out", bufs=2, space="PSUM") as ps_out, \
                 tc.tile_pool(name="ptp", bufs=4) as ptp, \
                 tc.tile_pool(name="normp", bufs=3) as normp, \
                 tc.tile_pool(name="ytfp", bufs=3) as ytfp, \
                 tc.tile_pool(name="osbp", bufs=3) as osbp:
                nc.sync.dma_start(
                    out=wo_sb, in_=woT.rearrange("(a p) o -> p a o", p=128))

                def attention_qb(qb):
                    b, p = qb // PQB, qb % PQB
                    qcols = slice(256 * qb, 256 * qb + 256)
                    qcols_hi = slice(256 * qb + 128, 256 * qb + 256)
                    nk = 2 * (p + 1)
                    po = ps_pv.tile([65, 4, 256], F32, tag="pv", name="po")
                    for kt in range(nk):
                        last = (kt == nk - 1)
                        kc = slice(b * Tt + 128 * kt, b * Tt + 128 * kt + 128)
                        ktg = b * (Tt // 128) + kt
                        sm = ps_att.tile([128, 4, 256], F32, tag="smega")
                        pt = ptp.tile([128, 4, 256], DT, tag="pt")
                        if not last:
                            for hp in range(2):
                                nc.tensor.matmul(
                                    sm[:, 2 * hp:2 * hp + 2, :],
                                    lhsT=kT_sb[:, kc],
                                    rhs=qT_sb[:, 2 * hp:2 * hp + 2, qcols],
                                    start=True, stop=True)
                            nc.scalar.activation(
                                pt, sm, mybir.ActivationFunctionType.Exp,
                                scale=0.125)
                            if kt == nk - 2:
                                for h in range(4):
                                    nc.vector.tensor_mul(
                                        pt[:, h, :], pt[:, h, :], m0_sb)
                            for hp in range(2):
                                nc.tensor.matmul(
                                    po[:, 2 * hp:2 * hp + 2, :],
                                    lhsT=v_sb[:, ktg, :],
                                    rhs=pt[:, 2 * hp:2 * hp + 2, :],
                                    start=(kt == 0), stop=(kt == nk - 2))
                        else:
                            # diagonal upper k-tile: only q-cols 128..256 live
                            for h in range(4):
                                nc.tensor.matmul(
                                    sm[:, h, 128:256],
                                    lhsT=kT_sb[:, kc],
                                    rhs=qT_sb[:, h, qcols_hi],
                                    start=True, stop=True)
                            nc.scalar.activation(
                                pt[:, :, 128:256], sm[:, :, 128:256],
                                mybir.ActivationFunctionType.Exp, scale=0.125)
                            for h in range(4):
                                nc.vector.tensor_mul(
                                    pt[:, h, 128:256], pt[:, h, 128:256],
                                    m0_sb[:, 0:128])
                            for h in range(4):
                                nc.tensor.matmul(
                                    po[:, h, 128:256],
                                    lhsT=v_sb[:, ktg, :],
                                    rhs=pt[:, h, 128:256],
                                    start=False, stop=True,
                                    skip_group_check=True)
                    ssb4 = normp.tile([1, 4, 256], F32, tag="ssb4")
                    nc.vector.tensor_copy(ssb4, po[64:65, :, :])
                    posb = normp.tile([64, 4, 256], F32, tag="posb")
                    nc.vector.tensor_copy(posb, po[0:64, :, :])
                    bca = normp.tile([64, 4, 256], F32, tag="bca")
                    nc.gpsimd.partition_broadcast(bca, ssb4)
                    rec = normp.tile([64, 4, 256], F32, tag="rec")
                    nc.vector.reciprocal_approx_fast(rec, bca)
                    nc.vector.tensor_mul(yloc[:, :, qcols], posb, rec)

                def gather(ch):
                    ccols = slice(512 * ch, 512 * ch + 512)
                    nc.sync.dma_start(
                        out=ytloc[ch].rearrange("(h d) t -> d h t", h=HPC),
                        in_=yloc[:, :, ccols])
                    nc.gpsimd.collective_compute(
                        "AllGather", mybir.AluOpType.bypass,
                        replica_groups=[list(range(NCORES))],
                        ins=[ytloc[ch]], outs=[ytful[ch]])

                def outproj(ch):
                    ytf = ytfp.tile([128, 16, 512], DT, tag="ytf",
                                    name=f"ytf{ch}")
                    for dt_ in range(16):
                        nc.sync.dma_start(
                            out=ytf[:, dt_, :],
                            in_=ytful[ch][128 * dt_:128 * dt_ + 128, :])
                    for ot in range(2):
                        pout = ps_out.tile([128, 512], F32, tag="out",
                                           name="pout")
                        for dt_ in range(16):
                            nc.tensor.matmul(
                                pout,
                                lhsT=wo_sb[:, dt_, 128 * ot:128 * ot + 128],
                                rhs=ytf[:, dt_, :],
                                start=(dt_ == 0), stop=(dt_ == 15))
                        ot_sb = osbp.tile([128, 512], F32, tag="osb")
                        nc.vector.tensor_copy(ot_sb, pout)
                        nc.sync.dma_start(
                            out=outp[128 * ot:128 * ot + 128,
                                     512 * ch:512 * ch + 512],
                            in_=ot_sb)

                order = ([0, 3, 1, 6, 4, 7, 5, 2] if NCH == 8
                         else list(range(NCH)))
                pend = []  # chunks gathered but not yet out-projected
                for i, ch in enumerate(order):
                    attention_qb(2 * ch)
                    attention_qb(2 * ch + 1)
                    gather(ch)
                    pend.append(ch)
                    if len(pend) >= 4:
                        outproj(pend.pop(0))
                for ch in pend:
                    outproj(ch)

    nc.compile()
    return nc


def host_inputs(x, cos, sin, wq, wk, wv, wo, Tt=T):
    """Build the 8 per-core input maps from full fp32 inputs."""
    BT = B * Tt
    x = np.asarray(x, np.float32)[:, :Tt, :]
    xT = np.ascontiguousarray(x.reshape(BT, DIM).T).astype(BF)

    cos = np.asarray(cos, np.float32)[:Tt]
    sin = np.asarray(sin, np.float32)[:Tt]
    cos2 = np.empty((128, BT), np.float32)
    sin2 = np.empty((128, BT), np.float32)
    for d in range(128):
        j = (d % 64) // 2
        cos2[d] = np.tile(cos[:, j], B)
        sin2[d] = np.tile(sin[:, j] if d % 2 else -sin[:, j], B)

    pswp = np.zeros((128, 128), BF)
    for i in range(128):
        pswp[i, i ^ 1] = 1
    ident = np.eye(64, dtype=BF)
    ii = np.arange(128)[:, None]
    jj = np.arange(256)[None, :]
    mask0 = (jj >= ii).astype(BF)
    mask1 = (jj >= ii + 128).astype(BF)

    maps = []
    for c in range(NCORES):
        qs = slice(256 * c, 256 * c + 256)
        ks = slice(64 * c, 64 * c + 64)
        wkv = np.concatenate([wk[ks], wv[ks]], axis=0)
        maps.append({
            "xT": xT,
            "wqT": np.ascontiguousarray(wq[qs].T).astype(BF),
            "wkvT": np.ascontiguousarray(wkv.T).astype(BF),
            "woT": np.ascontiguousarray(wo[qs].T).astype(BF),
            "cos2": cos2.astype(BF), "sin2": sin2.astype(BF),
            "pswp": pswp, "ident": ident,
            "mask0": mask0, "mask1": mask1,
        })
    return maps


_NC_CACHE = {}


def _get_nc(Tt=T):
    if Tt not in _NC_CACHE:
        _NC_CACHE[Tt] = build_nc(Tt)
    return _NC_CACHE[Tt]


def kernel(x, cos, sin, wq, wk, wv, wo):
    global LAST_RESULTS
    nc = _get_nc(T)
    maps = host_inputs(x, cos, sin, wq, wk, wv, wo)
    res = run_bass_kernel_spmd(nc, maps, core_ids=list(range(NCORES)))
    LAST_RESULTS = res
    out = np.empty((B * T, DIM), np.float32)
    for c in range(NCORES):
        out[:, 256 * c:256 * c + 256] = res.results[c]["out"].T
    return out.reshape(B, T, DIM)
